# Initial kernel scaffold
#
"""Trainium2 Bass kernel for nn_ExternalInteraction.

Computation (per batch b):
    img_sum[b, d]  = sum_i image[b, i, d]
    user_sum[b, d] = sum_u user[b, u, d]
    out_user[b, u, d] = user[b, u, d] * img_sum[b, d]
    out_img[b, i, d]  = image[b, i, d] * user_sum[b, d]

Sharding: data-parallel over batch B=64 across 8 cores (8 batches/core).

Per-core kernel strategy:
  - Load each batch's user [256,512] as an SBUF tile [128, 2, 512] and
    image [1024,512] as [128, 8, 512] via the contiguous "(p n) d" layout
    (each partition holds n consecutive rows -> maximally contiguous DMA).
  - Partition-axis reduce AND broadcast in one step on TensorE: matmul with
    an all-ones [128,128] stationary operand accumulates k-slices into a
    PSUM tile [128, 512] where EVERY partition holds the full column sum.
  - One VectorE multiply per output tensor (PSUM operand broadcast along
    the n free axis with a step-0 AP), then contiguous DMA back to DRAM.
"""

import numpy as np

B, U, I, D = 64, 256, 1024, 512
N_CORES = 8
BPC = B // N_CORES  # batches per core
P = 128
NU = U // P  # 2 user row-slices per batch
NI = I // P  # 8 image row-slices per batch

_STATE = {}


def _build():
    from contextlib import ExitStack

    import concourse.bass as bass
    import concourse.mybir as mybir
    from concourse.tile import TileContext

    nc = bass.Bass()
    f32 = mybir.dt.float32
    f32r = mybir.dt.float32r

    user_h = nc.declare_dram_parameter("user_attributes", [BPC, U, D], f32, isOutput=False)
    img_h = nc.declare_dram_parameter("image_attributes", [BPC, I, D], f32, isOutput=False)
    ouser_h = nc.declare_dram_parameter("out_user", [BPC, U, D], f32, isOutput=True)
    oimg_h = nc.declare_dram_parameter("out_img", [BPC, I, D], f32, isOutput=True)

    with TileContext(nc) as tc, ExitStack() as ctx:
        singles = ctx.enter_context(tc.tile_pool(name="singles", bufs=1))
        in_pool = ctx.enter_context(tc.tile_pool(name="in", bufs=3))
        out_pool = ctx.enter_context(tc.tile_pool(name="out", bufs=3))
        psum_pool = ctx.enter_context(tc.tile_pool(name="psum", bufs=4, space="PSUM"))

        ones = singles.tile([P, P], f32)
        nc.vector.memset(ones, 1.0)

        for b in range(BPC):
            usr_sb = in_pool.tile([P, NU, D], f32, tag="usr_in")
            img_sb = in_pool.tile([P, NI, D], f32, tag="img_in")
            nc.sync.dma_start(out=usr_sb, in_=user_h[b].rearrange("(p n) d -> p n d", p=P))
            nc.sync.dma_start(out=img_sb, in_=img_h[b].rearrange("(p n) d -> p n d", p=P))

            # Reduce over the partition axis and broadcast the result to all
            # 128 partitions in one accumulation group per tensor.
            usum = psum_pool.tile([P, D], f32, tag="usum")
            isum = psum_pool.tile([P, D], f32, tag="isum")
            for j in range(NU):
                nc.tensor.matmul(
                    usum,
                    ones.bitcast(f32r),
                    usr_sb[:, j, :].bitcast(f32r),
                    start=(j == 0),
                    stop=(j == NU - 1),
                )
            for j in range(NI):
                nc.tensor.matmul(
                    isum,
                    ones.bitcast(f32r),
                    img_sb[:, j, :].bitcast(f32r),
                    start=(j == 0),
                    stop=(j == NI - 1),
                )

            ousr_sb = out_pool.tile([P, NU, D], f32, tag="usr_out")
            oimg_sb = out_pool.tile([P, NI, D], f32, tag="img_out")
            nc.vector.tensor_mul(ousr_sb, usr_sb, isum.unsqueeze(1).broadcast_to([P, NU, D]))
            nc.vector.tensor_mul(oimg_sb, img_sb, usum.unsqueeze(1).broadcast_to([P, NI, D]))

            nc.sync.dma_start(out=ouser_h[b].rearrange("(p n) d -> p n d", p=P), in_=ousr_sb)
            nc.sync.dma_start(out=oimg_h[b].rearrange("(p n) d -> p n d", p=P), in_=oimg_sb)

    return nc


def _get_nc():
    if "nc" not in _STATE:
        _STATE["nc"] = _build()
    return _STATE["nc"]


def kernel(user_attributes, image_attributes):
    from concourse.bass_utils import run_bass_kernel_spmd

    user = np.ascontiguousarray(np.asarray(user_attributes, dtype=np.float32))
    img = np.ascontiguousarray(np.asarray(image_attributes, dtype=np.float32))

    nc = _get_nc()
    in_maps = [
        {
            "user_attributes": user[c * BPC : (c + 1) * BPC],
            "image_attributes": img[c * BPC : (c + 1) * BPC],
        }
        for c in range(N_CORES)
    ]
    res = run_bass_kernel_spmd(nc, in_maps, list(range(N_CORES)))
    out_user = np.concatenate([res.results[c]["out_user"] for c in range(N_CORES)], axis=0)
    out_img = np.concatenate([res.results[c]["out_img"] for c in range(N_CORES)], axis=0)
    return out_user, out_img


# revision 9
# speedup vs baseline: 1.1231x; 1.1231x over previous
"""Trainium2 Bass kernel for nn_ExternalInteraction.

Computation (per batch b):
    img_sum[b, d]  = sum_i image[b, i, d]
    user_sum[b, d] = sum_u user[b, u, d]
    out_user[b, u, d] = user[b, u, d] * img_sum[b, d]
    out_img[b, i, d]  = image[b, i, d] * user_sum[b, d]

Sharding: data-parallel over batch B=64 across 8 cores (8 batches/core).

Per-core kernel strategy:
  - Load each batch's user [256,512] as an SBUF tile [128, 2, 512] and
    image [1024,512] as [128, 8, 512] via the contiguous "(p n) d" layout
    (each partition holds n consecutive rows -> maximally contiguous DMA).
  - Partition-axis reduce AND broadcast in one step on TensorE: matmul with
    an all-ones [128,128] stationary operand accumulates k-slices into a
    PSUM tile [128, 512] where EVERY partition holds the full column sum.
  - One VectorE multiply per output tensor (PSUM operand broadcast along
    the n free axis with a step-0 AP), then contiguous DMA back to DRAM.
"""

import numpy as np

B, U, I, D = 64, 256, 1024, 512
N_CORES = 8
BPC = B // N_CORES  # batches per core
P = 128
NU = U // P  # 2 user row-slices per batch
NI = I // P  # 8 image row-slices per batch

_STATE = {}


def _build(repeats=1):
    from contextlib import ExitStack

    import concourse.bass as bass
    import concourse.mybir as mybir
    from concourse.tile import TileContext

    nc = bass.Bass()
    f32 = mybir.dt.float32

    user_h = nc.declare_dram_parameter("user_attributes", [BPC, U, D], f32, isOutput=False)
    img_h = nc.declare_dram_parameter("image_attributes", [BPC, I, D], f32, isOutput=False)
    ouser_h = nc.declare_dram_parameter("out_user", [BPC, U, D], f32, isOutput=True)
    oimg_h = nc.declare_dram_parameter("out_img", [BPC, I, D], f32, isOutput=True)

    with TileContext(nc) as tc, ExitStack() as ctx:
        singles = ctx.enter_context(tc.tile_pool(name="singles", bufs=1))
        in_pool = ctx.enter_context(tc.tile_pool(name="in", bufs=3))
        out_pool = ctx.enter_context(tc.tile_pool(name="out", bufs=3))
        psum_pool = ctx.enter_context(tc.tile_pool(name="psum", bufs=4, space="PSUM"))

        ones = singles.tile([P, P], f32)
        nc.vector.memset(ones, 1.0)

        for b in range(BPC * repeats):
            b = b % BPC
            usr_sb = in_pool.tile([P, NU, D], f32, tag="usr_in")
            img_sb = in_pool.tile([P, NI, D], f32, tag="img_in")
            nc.sync.dma_start(out=usr_sb, in_=user_h[b].rearrange("(p n) d -> p n d", p=P))
            nc.sync.dma_start(out=img_sb, in_=img_h[b].rearrange("(p n) d -> p n d", p=P))

            # Reduce over the partition axis and broadcast the result to all
            # 128 partitions in one accumulation group per tensor.
            usum = psum_pool.tile([P, D], f32, tag="usum")
            isum = psum_pool.tile([P, D], f32, tag="isum")
            for j in range(NU):
                nc.tensor.matmul(
                    usum,
                    ones,
                    usr_sb[:, j, :],
                    start=(j == 0),
                    stop=(j == NU - 1),
                )
            for j in range(NI):
                nc.tensor.matmul(
                    isum,
                    ones,
                    img_sb[:, j, :],
                    start=(j == 0),
                    stop=(j == NI - 1),
                )

            ousr_sb = out_pool.tile([P, NU, D], f32, tag="usr_out")
            oimg_sb = out_pool.tile([P, NI, D], f32, tag="img_out")
            nc.vector.tensor_mul(ousr_sb, usr_sb, isum.unsqueeze(1).broadcast_to([P, NU, D]))
            nc.vector.tensor_mul(oimg_sb, img_sb, usum.unsqueeze(1).broadcast_to([P, NI, D]))

            nc.sync.dma_start(out=ouser_h[b].rearrange("(p n) d -> p n d", p=P), in_=ousr_sb)
            nc.sync.dma_start(out=oimg_h[b].rearrange("(p n) d -> p n d", p=P), in_=oimg_sb)

    _spill_waits(nc)
    return nc


# Walrus's codegen allows only a small number of sync-wait commands on some
# engine instruction encodings (fused fp32 Matmult takes just 1; TensorTensor
# only slightly more), but the Tile scheduler can attach 2-3. Moving each
# wait onto its own NoOp directly before the instruction is semantically
# identical (the engine sequencer executes them in program order) and keeps
# every compute instruction within the encoding limit.
_SPILL_CLASSES = {
    "InstMatmult",
    "InstTensorTensor",
    "InstTensorReduce",
    "InstTensorCopy",
    "InstTensorScalar",
    "InstMemSet",
    "InstActivation",
    "InstDMACopy",
    "InstDrain",
}


def _spill_waits(nc):
    import concourse.mybir as mybir

    for fn in nc.m.functions:
        for blk in fn.blocks:
            new_list = []
            changed = False
            for inst in blk.instructions:
                si = inst.sync_info
                if (
                    inst.__class__.__name__ in _SPILL_CLASSES
                    and si is not None
                    and len(si.on_wait) >= 2
                ):
                    for k, w in enumerate(si.on_wait):
                        nop = mybir.InstNoOp(
                            name=f"{inst.name}-wnop{k}",
                            engine=inst.engine,
                            ins=[],
                            outs=[],
                        )
                        nop.bass_nofuse = True
                        nop.sync_info = mybir.SyncInfo(on_wait=[w], on_update=[])
                        new_list.append(nop)
                    inst.sync_info = mybir.SyncInfo(on_wait=[], on_update=list(si.on_update))
                    changed = True
                new_list.append(inst)
            if changed:
                blk.instructions = new_list


def _get_nc():
    if "nc" not in _STATE:
        _STATE["nc"] = _build()
    return _STATE["nc"]


def kernel(user_attributes, image_attributes):
    from concourse.bass_utils import run_bass_kernel_spmd

    user = np.ascontiguousarray(np.asarray(user_attributes, dtype=np.float32))
    img = np.ascontiguousarray(np.asarray(image_attributes, dtype=np.float32))

    nc = _get_nc()
    in_maps = [
        {
            "user_attributes": user[c * BPC : (c + 1) * BPC],
            "image_attributes": img[c * BPC : (c + 1) * BPC],
        }
        for c in range(N_CORES)
    ]
    res = run_bass_kernel_spmd(nc, in_maps, list(range(N_CORES)))
    out_user = np.concatenate([res.results[c]["out_user"] for c in range(N_CORES)], axis=0)
    out_img = np.concatenate([res.results[c]["out_img"] for c in range(N_CORES)], axis=0)
    return out_user, out_img
